# revision 3
# baseline (speedup 1.0000x reference)
"""GraphSelfAttentionLayer Trainium2 kernel.

Problem: B,N,F,H = 8,1024,1024,8 (HD=128). Data-parallel over B across the
8 NeuronCores (one batch element per core, weights replicated; no
collectives). Per core:

    q = obj @ Wq.T * 1/sqrt(HD)   (scale folded into Wq host-side)
    k = cross @ Wk.T ; v = cross @ Wv.T
    att_h = q_h @ k_h.T                      (per head, HD=128)
    A_u_h = exp(att_h) * expM                (expM = (adj>0)*exp(label_bias),
                                              host-precomputed multiplicative
                                              mask -- no -9e15 on device)
    S_h   = rowsum(A_u_h)  ; rs_h = 1/S_h
    out_h = (A_u_h @ (v @ Wo_h.T + bo_h)) * rs_h    (algebraic fusion:
                                              (A@v)@Wo.T == A@(v@Wo.T))
    att_avg = sum_h A_u_h * rs_h / H

All matmuls run in bf16 (fp32 PSUM accumulation). Softmax skips the rowmax
subtraction: scores are ~N(0, 0.41) so exp() is safely in range, and masked
entries are exact zeros via expM. Normalization is deferred past the AV
matmul and folded into the PSUM->SBUF copy as a per-partition scalar.
"""

import sys

sys.path.insert(0, "/opt/trn_rl_repo")

import numpy as np
import ml_dtypes

import concourse.bass as bass
import concourse.tile as tile
from concourse import bacc, mybir
from concourse.bass_utils import run_bass_kernel_spmd
from concourse.masks import make_identity

BF16 = mybir.dt.bfloat16
F32 = mybir.dt.float32
AF = mybir.ActivationFunctionType
ALU = mybir.AluOpType

P = 128
B, N, F, H = 8, 1024, 1024, 8
HD = F // H  # 128
CH = F // P  # 8 feature chunks
NCH = N // P  # 8 row chunks
NH = N // 512  # 2 free-dim halves

_PROG = None  # cached compiled Bass program


def _build_program():
    nc = bacc.Bacc("TRN2", target_bir_lowering=False, debug=False, num_devices=8)

    obj_d = nc.dram_tensor("obj", [N, F], F32, kind="ExternalInput")
    cross_d = nc.dram_tensor("cross", [N, F], F32, kind="ExternalInput")
    expm_d = nc.dram_tensor("expm", [N, N], BF16, kind="ExternalInput")
    wqt_d = nc.dram_tensor("wqt", [F, F], BF16, kind="ExternalInput")
    wkt_d = nc.dram_tensor("wkt", [F, F], BF16, kind="ExternalInput")
    wvt_d = nc.dram_tensor("wvt", [F, F], BF16, kind="ExternalInput")
    wot_d = nc.dram_tensor("wot", [F, F], BF16, kind="ExternalInput")
    bq_d = nc.dram_tensor("bq", [F], F32, kind="ExternalInput")
    bk_d = nc.dram_tensor("bk", [F], F32, kind="ExternalInput")
    bv_d = nc.dram_tensor("bv", [F], F32, kind="ExternalInput")
    bo_rep_d = nc.dram_tensor("bo_rep", [P, F], F32, kind="ExternalInput")
    out_d = nc.dram_tensor("out", [N, F], F32, kind="ExternalOutput")
    avg_d = nc.dram_tensor("att_avg", [N, N], F32, kind="ExternalOutput")

    with tile.TileContext(nc) as tc:
        with (
            tc.tile_pool(name="persist", bufs=1) as persist,
            tc.tile_pool(name="big", bufs=4) as big,
            tc.tile_pool(name="stage", bufs=2) as stage,
            tc.tile_pool(name="small", bufs=3) as small,
        ):
            ident = persist.tile([P, P], BF16, tag="ident")
            make_identity(nc, ident[:])

            qT = persist.tile([P, CH, N], BF16, tag="qT")
            kT = persist.tile([P, CH, N], BF16, tag="kT")
            vW = persist.tile([P, CH, F], BF16, tag="vW")
            expM = persist.tile([P, NCH, N], BF16, tag="expM")
            acc = persist.tile([P, NCH, N], BF16, tag="acc")
            bo_rep = persist.tile([P, F], F32, tag="bo_rep")
            bq_t = persist.tile([P, CH], F32, tag="bq")
            bk_t = persist.tile([P, CH], F32, tag="bk")
            bv_t = persist.tile([P, CH], F32, tag="bv")

            nc.sync.dma_start(bo_rep[:], bo_rep_d[:])
            nc.sync.dma_start(bq_t[:], bq_d.ap().rearrange("(o p) -> p o", p=P))
            nc.sync.dma_start(bk_t[:], bk_d.ap().rearrange("(o p) -> p o", p=P))
            nc.sync.dma_start(bv_t[:], bv_d.ap().rearrange("(o p) -> p o", p=P))
            nc.sync.dma_start(
                expM[:], expm_d.ap().rearrange("(no p) m -> p no m", p=P)
            )

            def load_w(dram):
                w = big.tile([P, CH, F], BF16, tag="big")
                nc.sync.dma_start(w[:], dram.ap().rearrange("(co p) f -> p co f", p=P))
                return w

            # ---- Phase A: input transposes, QKV projections, vW ----
            with (
                tc.tile_pool(name="psA", bufs=3, space="PSUM") as psA,
                tc.tile_pool(name="ptA", bufs=2, space="PSUM") as ptA,
            ):

                def transpose_in(x_dram):
                    """[N, F] f32 DRAM -> [P, CH, N] bf16 SBUF, feature-major."""
                    xT = big.tile([P, CH, N], BF16, tag="big")
                    for no in range(NCH):
                        stg = stage.tile([P, F], F32, tag="stg")
                        nc.sync.dma_start(stg[:], x_dram.ap()[no * P : (no + 1) * P, :])
                        cvt = stage.tile([P, F], BF16, tag="cvt")
                        nc.vector.tensor_copy(cvt[:], stg[:])
                        for cg in range(2):
                            pt = ptA.tile([P, 512], BF16, tag="ptA")
                            for j in range(4):
                                co = cg * 4 + j
                                nc.tensor.transpose(
                                    pt[:, j * P : (j + 1) * P],
                                    cvt[:, co * P : (co + 1) * P],
                                    ident[:],
                                )
                            nc.any.tensor_copy(
                                xT[:, cg * 4 : (cg + 1) * 4, no * P : (no + 1) * P],
                                pt[:].rearrange("p (c f) -> p c f", c=4),
                            )
                    return xT

                def project(dst, wT, srcT, bias_t):
                    for fo in range(CH):
                        for nh in range(NH):
                            ps = psA.tile([P, 512], F32, tag="psA")
                            for co in range(CH):
                                nc.tensor.matmul(
                                    ps[:],
                                    lhsT=wT[:, co, fo * P : (fo + 1) * P],
                                    rhs=srcT[:, co, nh * 512 : (nh + 1) * 512],
                                    start=(co == 0),
                                    stop=(co == CH - 1),
                                )
                            nc.scalar.activation(
                                dst[:, fo, nh * 512 : (nh + 1) * 512],
                                ps[:],
                                AF.Identity,
                                bias=bias_t[:, fo : fo + 1],
                            )

                wk = load_w(wkt_d)
                wv = load_w(wvt_d)
                crossT = transpose_in(cross_d)
                project(kT, wk, crossT, bk_t)
                vT = big.tile([P, CH, N], BF16, tag="big")
                project(vT, wv, crossT, bv_t)

                # vW[m, f'] = sum_f vT[f,m] * WoT[f,f'] + bo[f']
                wo = load_w(wot_d)
                for mo in range(CH):
                    for fh in range(NH):
                        ps = psA.tile([P, 512], F32, tag="psA")
                        for fo in range(CH):
                            nc.tensor.matmul(
                                ps[:],
                                lhsT=vT[:, fo, mo * P : (mo + 1) * P],
                                rhs=wo[:, fo, fh * 512 : (fh + 1) * 512],
                                start=(fo == 0),
                                stop=(fo == CH - 1),
                            )
                        nc.vector.tensor_add(
                            vW[:, mo, fh * 512 : (fh + 1) * 512],
                            ps[:],
                            bo_rep[:, fh * 512 : (fh + 1) * 512],
                        )

                wq = load_w(wqt_d)
                objT = transpose_in(obj_d)
                project(qT, wq, objT, bq_t)

            # ---- Phase B: per-head attention (software-pipelined) ----
            with (
                tc.tile_pool(name="psatt", bufs=4, space="PSUM") as psatt,
                tc.tile_pool(name="pst", bufs=2, space="PSUM") as pst,
                tc.tile_pool(name="psav", bufs=2, space="PSUM") as psav,
            ):
                st = {}  # per-head stage-1 products

                def stage1(h):
                    A_u = big.tile([P, NCH, N], BF16, tag="big")
                    S = small.tile([P, NCH], F32, tag="S")
                    for no in range(NCH):
                        ex = stage.tile([P, N], BF16, tag="exp")
                        for mh in range(NH):
                            pa = psatt.tile([P, 512], F32, tag="att")
                            nc.tensor.matmul(
                                pa[:],
                                lhsT=qT[:, h, no * P : (no + 1) * P],
                                rhs=kT[:, h, mh * 512 : (mh + 1) * 512],
                                start=True,
                                stop=True,
                            )
                            nc.scalar.activation(
                                ex[:, mh * 512 : (mh + 1) * 512], pa[:], AF.Exp
                            )
                        nc.vector.scalar_tensor_tensor(
                            out=A_u[:, no, :],
                            in0=ex[:],
                            scalar=1.0,
                            in1=expM[:, no, :],
                            op0=ALU.mult,
                            op1=ALU.mult,
                            accum_out=S[:, no : no + 1],
                        )
                    rs = small.tile([P, NCH], F32, tag="rs")
                    rs8 = small.tile([P, NCH], F32, tag="rs8")
                    nc.vector.reciprocal(rs[:], S[:])
                    nc.vector.tensor_scalar_mul(rs8[:], rs[:], 1.0 / H)
                    st[h] = (A_u, rs, rs8)

                def stage2(h):
                    A_u, rs, rs8 = st.pop(h)
                    A_uT = big.tile([P, NCH, N], BF16, tag="big")
                    for mo in range(CH):
                        for ng in range(NH):
                            pt = pst.tile([P, 512], BF16, tag="pt")
                            for j in range(4):
                                no = ng * 4 + j
                                nc.tensor.transpose(
                                    pt[:, j * P : (j + 1) * P],
                                    A_u[:, no, mo * P : (mo + 1) * P],
                                    ident[:],
                                )
                            nc.any.tensor_copy(
                                A_uT[:, mo, ng * 512 : (ng + 1) * 512], pt[:]
                            )
                    for no in range(NCH):
                        pav = psav.tile([P, HD], F32, tag="av")
                        for mo in range(CH):
                            nc.tensor.matmul(
                                pav[:],
                                lhsT=A_uT[:, mo, no * P : (no + 1) * P],
                                rhs=vW[:, mo, h * HD : (h + 1) * HD],
                                start=(mo == 0),
                                stop=(mo == CH - 1),
                            )
                        ot = small.tile([P, HD], F32, tag="ot")
                        nc.vector.tensor_scalar_mul(ot[:], pav[:], rs[:, no : no + 1])
                        nc.sync.dma_start(
                            out_d.ap()[no * P : (no + 1) * P, h * HD : (h + 1) * HD],
                            ot[:],
                        )
                    for no in range(NCH):
                        if h == 0:
                            nc.vector.tensor_scalar_mul(
                                acc[:, no, :], A_u[:, no, :], rs8[:, no : no + 1]
                            )
                        else:
                            nc.vector.scalar_tensor_tensor(
                                out=acc[:, no, :],
                                in0=A_u[:, no, :],
                                scalar=rs8[:, no : no + 1],
                                in1=acc[:, no, :],
                                op0=ALU.mult,
                                op1=ALU.add,
                            )

                for h in range(H):
                    stage1(h)
                    if h > 0:
                        stage2(h - 1)
                stage2(H - 1)

            # ---- Phase C: att_avg out ----
            for no in range(NCH):
                cv = stage.tile([P, N], F32, tag="cvf")
                nc.vector.tensor_copy(cv[:], acc[:, no, :])
                nc.sync.dma_start(avg_d.ap()[no * P : (no + 1) * P, :], cv[:])

    nc.compile()
    return nc


def _get_program():
    global _PROG
    if _PROG is None:
        _PROG = _build_program()
    return _PROG


def kernel(
    obj_feats,
    cross_feats,
    adj_matrix,
    label_biases_att,
    Wq,
    bq,
    Wk,
    bk,
    Wv,
    bv,
    Wo,
    bo,
):
    obj_feats = np.asarray(obj_feats, np.float32)
    cross_feats = np.asarray(cross_feats, np.float32)
    adj_matrix = np.asarray(adj_matrix)
    label_biases_att = np.asarray(label_biases_att, np.float32)
    Wq = np.asarray(Wq, np.float32)
    Wk = np.asarray(Wk, np.float32)
    Wv = np.asarray(Wv, np.float32)
    Wo = np.asarray(Wo, np.float32)
    bq = np.asarray(bq, np.float32)
    bk = np.asarray(bk, np.float32)
    bv = np.asarray(bv, np.float32)
    bo = np.asarray(bo, np.float32)

    bf16 = ml_dtypes.bfloat16
    s = np.float32(1.0 / np.sqrt(HD))
    wqt = np.ascontiguousarray((Wq.T * s).astype(bf16))  # [C, F], scale folded
    wkt = np.ascontiguousarray(Wk.T.astype(bf16))
    wvt = np.ascontiguousarray(Wv.T.astype(bf16))
    # WoT[f, h*HD+hd] = Wo[h, hd, f]
    wot = np.ascontiguousarray(Wo.transpose(2, 0, 1).reshape(F, F).astype(bf16))
    bo_rep = np.ascontiguousarray(np.broadcast_to(bo, (P, F)).astype(np.float32))
    bq_s = (bq * s).astype(np.float32)

    # multiplicative mask: exp(label_bias) where adj>0 else 0
    expm = np.where(adj_matrix > 0, np.exp(label_biases_att), np.float32(0.0)).astype(
        bf16
    )

    nc = _get_program()
    in_maps = []
    for b in range(B):
        in_maps.append(
            {
                "obj": np.ascontiguousarray(obj_feats[b]),
                "cross": np.ascontiguousarray(cross_feats[b]),
                "expm": np.ascontiguousarray(expm[b]),
                "wqt": wqt,
                "wkt": wkt,
                "wvt": wvt,
                "wot": wot,
                "bq": bq_s,
                "bk": bk,
                "bv": bv,
                "bo_rep": bo_rep,
            }
        )
    res = run_bass_kernel_spmd(nc, in_maps, core_ids=list(range(B)))
    out = np.stack([res.results[b]["out"] for b in range(B)])
    att_avg = np.stack([res.results[b]["att_avg"] for b in range(B)])
    return out, att_avg
